# revision 9
# baseline (speedup 1.0000x reference)
"""GCN (2-layer, PyG gcn_norm) on 8 Trainium2 NeuronCores via Bass.

Strategy (dst-range sharding, no collectives):
  * Host sorts edges by dst and lays them out into per-node-tile slots
    (128-node tiles, padded to S chunks of 128 edge-lanes per tile).
  * Device sweep = for each node tile: one-hot dst masks (built on DVE from
    relative dst values) are the PE moving tensor; per-edge messages are the
    stationary tensor, split into bf16 hi/lo columns so the fp32 value is
    reconstructed exactly in PSUM accumulation (error ~2^-16 relative).
  * Three sequential NEFF launches: (1) deg -> dinv, (2) layer-1 aggregation
    -> h -> v, (3) layer-2 aggregation -> output. Between launches the host
    only performs index-space stream building (gather of returned per-node
    arrays into per-edge streams).
"""

import os
import sys

sys.path.insert(0, "/opt/trn_rl_repo")

import numpy as np
import ml_dtypes

import concourse.bass as bass
import concourse.tile as tile
from concourse import mybir
from concourse.bass_utils import run_bass_kernel_spmd

BF16 = ml_dtypes.bfloat16

N = 100000
E = 3200000
D = 2
HID = 16
NCORE = 8
TPC = 98                      # node tiles per core
TOTAL_TILES = NCORE * TPC     # 784
NPAD = TOTAL_TILES * 128      # 100352
NSH = TPC * 128               # 12544 nodes per core
MASK_BATCH = 8                # chunks per mask-build instruction
G = 8                         # same-dst edges pre-reduced per lane (DVE)


def _split_multi_waits(nc):
    """This toolchain's walrus encodes at most one sync-wait per instruction.
    Hoist extra waits onto fresh single-wait NoOps placed just before."""
    ctr = 0
    for fn in nc.m.functions:
        for bb in fn.blocks:
            insts = list(bb.instructions)
            if not any(
                i.sync_info is not None and len(i.sync_info.on_wait or []) > 1
                for i in insts
            ):
                continue
            new = []
            for inst in insts:
                si = inst.sync_info
                if si is not None and len(si.on_wait or []) > 1:
                    waits = list(si.on_wait)
                    for w in waits[:-1]:
                        ctr += 1
                        new.append(
                            mybir.InstNoOp(
                                name=f"wsplit-{ctr}",
                                engine=inst.engine,
                                sync_info=mybir.SyncInfo(on_wait=[w], on_update=[]),
                                bass_nofuse=True,
                            )
                        )
                    si.on_wait = [waits[-1]]
                new.append(inst)
            bb.instructions = new
    return ctr


def _preprocess(edge_index, edge_weight):
    """Sort edges by dst; group each node's edges into lanes of G (device
    pre-reduces the G slots of a lane on DVE before the PE scatter).
    Edge slot layout: (tile, chunk, partition-lane, g); lane layout for the
    per-lane dst-relative id: (tile, chunk, partition-lane)."""
    src = edge_index[0].astype(np.int64)
    dst = edge_index[1].astype(np.int64)
    perm = np.argsort(dst, kind="stable")
    src_s = src[perm]
    dst_s = dst[perm]
    ew_s = edge_weight[perm].astype(np.float32)

    c = np.bincount(dst_s, minlength=NPAD)            # per-node degree
    L = (c + G - 1) // G                              # lanes per node
    Lt = L.reshape(TOTAL_TILES, 128)
    S = int(np.ceil(Lt.sum(axis=1).max() / 128))      # lane chunks per tile
    lane_off = np.zeros_like(Lt)
    lane_off[:, 1:] = np.cumsum(Lt[:, :-1], axis=1)
    lane_base = lane_off.reshape(NPAD)                # node's first lane in tile

    node_start = np.zeros(NPAD + 1, np.int64)
    np.cumsum(c, out=node_start[1:])
    r = np.arange(len(dst_s)) - node_start[dst_s]
    lane = lane_base[dst_s] + r // G                  # lane within tile
    g = r % G
    t = dst_s >> 7
    lane_slot = (t * S + lane // 128) * 128 + (lane % 128)
    slot = lane_slot * G + g

    tot = TOTAL_TILES * S * 128
    ew_pad = np.zeros(tot * G, np.float32)
    src_pad = np.zeros(tot * G, np.int64)
    dstrel_pad = np.zeros(tot, np.float32)
    ew_pad[slot] = ew_s
    src_pad[slot] = src_s
    dstrel_pad[lane_slot] = (dst_s & 127).astype(np.float32)
    return dict(S=S, ew=ew_pad, src=src_pad, dstrel=dstrel_pad)


def _to_core_stream(arr, S, dtype, inner=1):
    """[TOTAL_TILES*S*128*inner] slot array -> per-core [128, TPC*S*inner]."""
    a = arr.reshape(TOTAL_TILES, S, 128, inner)
    out = []
    for c in range(NCORE):
        blk = a[c * TPC:(c + 1) * TPC]            # [TPC, S, 128, inner]
        out.append(np.ascontiguousarray(
            blk.transpose(2, 0, 1, 3).reshape(128, TPC * S * inner)).astype(dtype))
    return out


def _to_core_nodes(arr, dtype):
    """[NPAD] node array -> per-core [128, TPC] (node = c*NSH + t*128 + p)."""
    a = arr.reshape(TOTAL_TILES, 128)
    out = []
    for c in range(NCORE):
        blk = a[c * TPC:(c + 1) * TPC]            # [TPC, 128]
        out.append(np.ascontiguousarray(blk.T).astype(dtype))
    return out


def _from_core_nodes(parts):
    """inverse of _to_core_nodes -> [NPAD] float32."""
    full = np.empty((TOTAL_TILES, 128), np.float32)
    for c in range(NCORE):
        full[c * TPC:(c + 1) * TPC] = np.asarray(parts[c], np.float32).T
    return full.reshape(NPAD)


def _mask_build(nc, mask_t, dstrel_t, iota_sb, s0, nb):
    """mask_t[:, 0:nb*128] (bf16) = one-hot of dstrel_t[:, s0:s0+nb] vs iota."""
    in0 = dstrel_t[:, s0:s0 + nb].unsqueeze(2).broadcast_to([128, nb, 128])
    in1 = iota_sb[:, :].unsqueeze(1).broadcast_to([128, nb, 128])
    out = mask_t[:, 0:nb * 128].rearrange("p (s n) -> p s n", n=128)
    nc.vector.tensor_tensor(out, in0, in1, mybir.AluOpType.is_equal)


def _hilo(nc, pool, S, msrc, mt_view_hi, mt_view_lo):
    """Write bf16 hi/lo split of f32 msrc into (strided) bf16 views."""
    nc.vector.tensor_copy(mt_view_hi, msrc)                      # f32 -> bf16
    hif = pool.tile([128, S], mybir.dt.float32, tag="hif")
    nc.vector.tensor_copy(hif, mt_view_hi)                       # bf16 -> f32
    lof = pool.tile([128, S], mybir.dt.float32, tag="lof")
    nc.vector.tensor_sub(lof, msrc, hif)
    nc.vector.tensor_copy(mt_view_lo, lof)                       # f32 -> bf16


def _build_sweep(mode, S, tpc=TPC, reps=1, skip_masks=False, skip_mm=False):
    """Build the Bass program for one sweep. mode in {deg, layer1, layer2}.
    reps>1 wraps the main tile loop in a hardware For_i loop repeating the
    identical (idempotent) work — used only for timing measurements."""
    from contextlib import ExitStack

    F = D if mode == "layer1" else 1
    M = 2 * F  # stationary columns (hi.., lo..)
    CS = tpc * S
    CSG = tpc * S * G
    f32 = mybir.dt.float32
    bf = mybir.dt.bfloat16

    nc = bass.Bass("TRN2", target_bir_lowering=False, debug=False,
                   num_devices=NCORE)
    dram = {}

    def din(name, shape, dtype=f32):
        dram[name] = nc.dram_tensor(name, shape, dtype, kind="ExternalInput").ap()
        return dram[name]

    def dout(name, shape, dtype=f32):
        dram[name] = nc.dram_tensor(name, shape, dtype, kind="ExternalOutput").ap()
        return dram[name]

    iota_d = din("iota", [128, 128], bf)
    ident_d = din("ident", [128, 128])
    dstrel_d = din("dstrel", [128, CS], bf)
    ew_d = din("ew", [128, CSG])
    if mode == "layer1":
        dinvsrc_d = din("dinvsrc", [128, CSG])
        xs_d = [din(f"xsrc{f}", [128, CSG]) for f in range(D)]
        dinv_d = din("dinvn", [128, tpc])
        xn_d = [din(f"xn{f}", [128, tpc]) for f in range(D)]
        w1_d = din("w1b", [128, 2 * HID])
        b1_d = din("b1b", [128, HID])
        w2_d = din("w2b", [128, HID])
        v_out = dout("vout", [128, tpc])
    elif mode == "layer2":
        vsrc_d = din("vsrc", [128, CSG])
        dinv_d = din("dinvn", [128, tpc])
        vn_d = din("vn", [128, tpc])
        b2_d = din("b2b", [128, 1])
        y_out = dout("yout", [128, tpc])
    else:
        dinv_out = dout("dinvout", [128, tpc])

    with tile.TileContext(nc) as tc, ExitStack() as ctx:
        const = ctx.enter_context(tc.tile_pool(name="const", bufs=1))
        sp = ctx.enter_context(tc.tile_pool(name="streams", bufs=3))
        wp = ctx.enter_context(tc.tile_pool(name="work", bufs=3))
        mp = ctx.enter_context(tc.tile_pool(name="masks", bufs=3))
        accp = ctx.enter_context(tc.tile_pool(name="acc", bufs=1))
        psA = ctx.enter_context(tc.tile_pool(name="psA", bufs=4, space="PSUM"))
        psB = ctx.enter_context(tc.tile_pool(name="psB", bufs=2, space="PSUM"))

        iota_sb = const.tile([128, 128], bf)
        nc.sync.dma_start(iota_sb[:], iota_d[:])
        ident_sb = const.tile([128, 128], f32)
        nc.sync.dma_start(ident_sb[:], ident_d[:])

        aggN = accp.tile([128, tpc * F], f32)  # col = t*F + f

        if mode == "layer1":
            dinvN = const.tile([128, tpc], f32)
            nc.sync.dma_start(dinvN[:], dinv_d[:])
            xN = []
            for f in range(D):
                t_ = const.tile([128, tpc], f32, tag=f"xn{f}")
                nc.sync.dma_start(t_[:], xn_d[f][:])
                xN.append(t_)
            w1_sb = const.tile([128, 2 * HID], f32)
            nc.sync.dma_start(w1_sb[:], w1_d[:])
            b1_sb = const.tile([128, HID], f32)
            nc.sync.dma_start(b1_sb[:], b1_d[:])
            w2_sb = const.tile([128, HID], f32)
            nc.sync.dma_start(w2_sb[:], w2_d[:])
        elif mode == "layer2":
            dinvN = const.tile([128, tpc], f32)
            nc.sync.dma_start(dinvN[:], dinv_d[:])
            vN = const.tile([128, tpc], f32)
            nc.sync.dma_start(vN[:], vn_d[:])
            b2_sb = const.tile([128, 1], f32)
            nc.sync.dma_start(b2_sb[:], b2_d[:])

        def _reduce_g(dst, src_f32):
            nc.vector.tensor_reduce(
                dst.unsqueeze(2), src_f32.rearrange("p (s g) -> p s g", g=G),
                mybir.AxisListType.X, mybir.AluOpType.add)

        def _tile_loop():
          for t in range(tpc):
            c0 = t * S
            cg0 = t * S * G
            ew_t = sp.tile([128, S * G], f32, tag="ew")
            nc.sync.dma_start(ew_t[:], ew_d[:, cg0:cg0 + S * G])
            dstrel_t = sp.tile([128, S], bf, tag="dstrel")
            nc.sync.dma_start(dstrel_t[:], dstrel_d[:, c0:c0 + S])

            # stationary tensor [128, M*S]; chunk s occupies cols M*s..M*s+M
            # with column order (hi_0..hi_{F-1}, lo_0..lo_{F-1})
            mt = wp.tile([128, M * S], bf, tag="mt")
            mtv = mt.rearrange("p (s k) -> p k s", k=M)

            if mode == "deg":
                red = wp.tile([128, S], f32, tag="red0")
                _reduce_g(red, ew_t)
                _hilo(nc, wp, S, red, mtv[:, 0], mtv[:, 1])
            elif mode == "layer1":
                dsrc_t = sp.tile([128, S * G], f32, tag="dsrc")
                nc.sync.dma_start(dsrc_t[:], dinvsrc_d[:, cg0:cg0 + S * G])
                t1 = wp.tile([128, S * G], f32, tag="t1")
                nc.vector.tensor_mul(t1, ew_t, dsrc_t)
                for f in range(D):
                    xs_t = sp.tile([128, S * G], f32, tag=f"xs{f}")
                    nc.sync.dma_start(xs_t[:], xs_d[f][:, cg0:cg0 + S * G])
                    mf = wp.tile([128, S * G], f32, tag=f"mf{f}")
                    nc.vector.tensor_mul(mf, t1, xs_t)
                    red = wp.tile([128, S], f32, tag=f"red{f}")
                    _reduce_g(red, mf)
                    _hilo(nc, wp, S, red, mtv[:, f], mtv[:, F + f])
            else:
                vs_t = sp.tile([128, S * G], f32, tag="vs")
                nc.sync.dma_start(vs_t[:], vsrc_d[:, cg0:cg0 + S * G])
                mf = wp.tile([128, S * G], f32, tag="mf0")
                nc.vector.tensor_mul(mf, ew_t, vs_t)
                red = wp.tile([128, S], f32, tag="red0")
                _reduce_g(red, mf)
                _hilo(nc, wp, S, red, mtv[:, 0], mtv[:, 1])

            psum_t = psA.tile([M, 128], f32, tag="psum")
            s = 0
            while s < S:
                nb = min(MASK_BATCH, S - s)
                mask_t = mp.tile([128, MASK_BATCH * 128], bf, tag="mask")
                if not skip_masks or (t == 0 and s == 0):
                    _mask_build(nc, mask_t, dstrel_t, iota_sb, s, nb)
                for k in range(nb):
                    if skip_mm and not (s + k == 0 or s + k == S - 1):
                        continue
                    nc.tensor.matmul(
                        psum_t[:, :],
                        lhsT=mt[:, M * (s + k):M * (s + k) + M],
                        rhs=mask_t[:, k * 128:(k + 1) * 128],
                        start=(s + k == 0),
                        stop=(s + k == S - 1),
                    )
                s += nb

            # transpose [M,128] -> [128,M] and combine hi+lo into aggN
            zt = wp.tile([M, 128], f32, tag="zt")
            nc.vector.tensor_copy(zt, psum_t)
            pt2 = psB.tile([128, M], f32, tag="pt2")
            nc.tensor.matmul(pt2[:, :], lhsT=zt[:, :],
                             rhs=ident_sb[0:M, 0:M], is_transpose=True)
            ptsb = wp.tile([128, M], f32, tag="ptsb")
            nc.vector.tensor_copy(ptsb, pt2)
            nc.vector.tensor_add(aggN[:, t * F:(t + 1) * F],
                                 ptsb[:, 0:F], ptsb[:, F:M])

        if reps > 1:
            with tc.For_i(0, reps, 1):
                _tile_loop()
        else:
            _tile_loop()

        # ---- epilogue ----
        agf = aggN.rearrange("p (t f) -> p f t", f=F)
        if mode == "deg":
            deg = wp.tile([128, tpc], f32, tag="deg")
            nc.vector.tensor_scalar(deg, agf[:, 0], 1.0, None,
                                    mybir.AluOpType.add)
            sq = wp.tile([128, tpc], f32, tag="sq")
            nc.scalar.activation(sq, deg, mybir.ActivationFunctionType.Sqrt)
            dinv = wp.tile([128, tpc], f32, tag="dinvr")
            nc.vector.reciprocal(dinv, sq)
            nc.sync.dma_start(dinv_out[:], dinv[:])
        elif mode == "layer1":
            dsq = wp.tile([128, tpc], f32, tag="dsq")
            nc.vector.tensor_mul(dsq, dinvN, dinvN)
            zf = []
            for f in range(D):
                a = wp.tile([128, tpc], f32, tag=f"za{f}")
                nc.vector.tensor_mul(a, agf[:, f], dinvN)
                b = wp.tile([128, tpc], f32, tag=f"zb{f}")
                nc.vector.tensor_mul(b, xN[f], dsq)
                z = accp.tile([128, tpc], f32, tag=f"z{f}")
                nc.vector.tensor_add(z, a, b)
                zf.append(z)
            h_sb = accp.tile([128, HID * tpc], f32)
            for j in range(HID):
                hj = h_sb[:, j * tpc:(j + 1) * tpc]
                a = wp.tile([128, tpc], f32, tag="ha")
                nc.vector.tensor_scalar(a, zf[0], w1_sb[:, j:j + 1], None,
                                        mybir.AluOpType.mult)
                b = wp.tile([128, tpc], f32, tag="hb")
                nc.vector.tensor_scalar(b, zf[1], w1_sb[:, HID + j:HID + j + 1],
                                        None, mybir.AluOpType.mult)
                nc.vector.tensor_add(hj, a, b)
                nc.vector.tensor_scalar(hj, hj, b1_sb[:, j:j + 1], None,
                                        mybir.AluOpType.add)
            nc.scalar.activation(h_sb, h_sb, mybir.ActivationFunctionType.Relu)
            vacc = wp.tile([128, tpc], f32, tag="vacc")
            nc.vector.tensor_scalar(vacc, h_sb[:, 0:tpc], w2_sb[:, 0:1], None,
                                    mybir.AluOpType.mult)
            for j in range(1, HID):
                tmp = wp.tile([128, tpc], f32, tag="vtmp")
                nc.vector.tensor_scalar(tmp, h_sb[:, j * tpc:(j + 1) * tpc],
                                        w2_sb[:, j:j + 1], None,
                                        mybir.AluOpType.mult)
                nc.vector.tensor_add(vacc, vacc, tmp)
            vres = wp.tile([128, tpc], f32, tag="vres")
            nc.vector.tensor_mul(vres, vacc, dinvN)
            nc.sync.dma_start(v_out[:], vres[:])
        else:
            a = wp.tile([128, tpc], f32, tag="ya")
            nc.vector.tensor_mul(a, agf[:, 0], dinvN)
            b = wp.tile([128, tpc], f32, tag="yb")
            nc.vector.tensor_mul(b, vN, dinvN)
            y = wp.tile([128, tpc], f32, tag="y")
            nc.vector.tensor_add(y, a, b)
            nc.vector.tensor_scalar(y, y, b2_sb[:, 0:1], None,
                                    mybir.AluOpType.add)
            nc.sync.dma_start(y_out[:], y[:])

    _split_multi_waits(nc)
    return nc


_IOTA = np.tile(np.arange(128, dtype=np.float32).astype(BF16), (128, 1))
_IDENT = np.eye(128, dtype=np.float32)


def kernel(x, edge_index, edge_weight, W1, b1, W2, b2, _timing=None):
    x = np.asarray(x, np.float32)
    edge_index = np.asarray(edge_index)
    edge_weight = np.asarray(edge_weight, np.float32)
    W1 = np.asarray(W1, np.float32)
    b1 = np.asarray(b1, np.float32)
    W2 = np.asarray(W2, np.float32)
    b2 = np.asarray(b2, np.float32)

    pp = _preprocess(edge_index, edge_weight)
    S = pp["S"]

    xp = np.zeros((NPAD, D), np.float32)
    xp[:N] = x

    ew_cs = _to_core_stream(pp["ew"], S, np.float32, inner=G)
    dstrel_cs = _to_core_stream(pp["dstrel"], S, BF16)

    common = {"iota": np.ascontiguousarray(_IOTA),
              "ident": np.ascontiguousarray(_IDENT)}

    # ---- NEFF 1: deg -> dinv ----
    nc1 = _build_sweep("deg", S)
    in1 = [dict(common, dstrel=dstrel_cs[c], ew=ew_cs[c]) for c in range(NCORE)]
    r1 = run_bass_kernel_spmd(nc1, in1, core_ids=list(range(NCORE)))
    dinv = _from_core_nodes([r1.results[c]["dinvout"] for c in range(NCORE)])

    # ---- host glue: per-edge dinv[src], x[src] streams ----
    dinvsrc_cs = _to_core_stream(dinv[pp["src"]], S, np.float32, inner=G)
    xs_cs = [_to_core_stream(xp[pp["src"], f], S, np.float32, inner=G)
             for f in range(D)]
    dinv_n = _to_core_nodes(dinv, np.float32)
    xn = [_to_core_nodes(xp[:, f], np.float32) for f in range(D)]
    w1b = np.tile(W1.reshape(1, 2 * HID), (128, 1)).astype(np.float32)
    b1b = np.tile(b1.reshape(1, HID), (128, 1)).astype(np.float32)
    w2b = np.tile(W2.reshape(1, HID), (128, 1)).astype(np.float32)
    b2b = np.full((128, 1), float(b2[0]), np.float32)

    # ---- NEFF 2: layer 1 -> v ----
    nc2 = _build_sweep("layer1", S)
    in2 = [dict(common, dstrel=dstrel_cs[c], ew=ew_cs[c],
                dinvsrc=dinvsrc_cs[c], xsrc0=xs_cs[0][c], xsrc1=xs_cs[1][c],
                dinvn=dinv_n[c], xn0=xn[0][c], xn1=xn[1][c],
                w1b=w1b, b1b=b1b, w2b=w2b) for c in range(NCORE)]
    r2 = run_bass_kernel_spmd(nc2, in2, core_ids=list(range(NCORE)))
    v = _from_core_nodes([r2.results[c]["vout"] for c in range(NCORE)])

    # ---- host glue: v[src] stream ----
    vsrc_cs = _to_core_stream(v[pp["src"]], S, np.float32, inner=G)
    vn = _to_core_nodes(v, np.float32)

    # ---- NEFF 3: layer 2 -> output ----
    nc3 = _build_sweep("layer2", S)
    in3 = [dict(common, dstrel=dstrel_cs[c], ew=ew_cs[c], vsrc=vsrc_cs[c],
                dinvn=dinv_n[c], vn=vn[c], b2b=b2b) for c in range(NCORE)]
    r3 = run_bass_kernel_spmd(nc3, in3, core_ids=list(range(NCORE)))
    y = _from_core_nodes([r3.results[c]["yout"] for c in range(NCORE)])

    return y[:N, None].astype(np.float32)
